# revision 1
# baseline (speedup 1.0000x reference)
"""Trainium2 Bass kernel for nn_BGNLLLoss (person-partition layout).

Math per element (t,p), derived from the bivariate-Gaussian NLL:
    a = (x-mux)e^{-lsx},  b = (y-muy)e^{-lsy},  u = 2*pc
    nll_raw = (1+e^{-u})*( sq(e^{pc}(a-b)) + sq(a+b) )/8
              + lsx + lsy - pc - ln(1+e^{-u}) + ln2 + ln(2pi)
    nll = min(nll_raw, -ln(1e-20));   loss[p] = sum_t nll
(identity used: (e^u+1)sq(m) + (e^{-u}+1)sq(p) = (1+e^{-u})(sq(e^{pc}m)+sq(p)))

Layout: persons on partitions (4 groups x 128 per core), frames on the free
dim (2 windows x 2048). Host pre-transposes/casts the 7 input planes to bf16,
ordered [x, muy, mux, y, lsx, lsy, pc] so one paired tensor_sub gives
[nx | -ny] and one paired tensor_mul gives [a | -b].

Engine split per block:
  ScalarE: [isx|isy] = Exp(-[lsx|lsy]-1.5ln2) (one paired ACTIVATE),
           E1 = e^{pc}, GM = e^{-2pc}                      (3 ACTIVATEs)
  GpSimd : s1m = lsx+lsy-pc (STT+add), HF = GM+1 (f32),
           lvc = (bits(HF)-SIGMA)*(-LNK)  [fast-log], s1b = s1m+lvc
  VectorE: paired sub/mul, m/p2, mq, SQ2 custom, W = (GM+1)*Q via STT,
           V = W+s1b, min+accum ride (tensor_scalar accum_out)
Frame-sum: accum_out per (group, window) column; final tiny adds + one DMA.
Sharding: person dim across 8 cores (512 each), no collectives.
"""

import math
from contextlib import ExitStack

import numpy as np

import concourse.bass as bass
import concourse.bacc as bacc
import concourse.mybir as mybir
import concourse.tile as tile
from concourse import bass_utils
from concourse.dve_spec import (
    Spec, Src0, Src1, C0, Zero, AluOp, lower, sq, minn, _has_src1,
)
from concourse.dve_uop import DveOpSpec
import concourse.dve_ops as dve_ops

F32 = mybir.dt.float32
BF16 = mybir.dt.bfloat16
I32 = mybir.dt.int32
AF = mybir.ActivationFunctionType
ALU = mybir.AluOpType

T = 4096
P = 4096
N_CORES = 8
PCC = P // N_CORES         # persons per core = 512
G = PCC // 128             # partition groups = 4
F = 2048                   # frames per window
NF = T // F                # windows = 2
NB = G * NF                # blocks per core = 8

LN2 = math.log(2.0)
LOG2PI = math.log(2.0 * math.pi)
CLAMP = -math.log(1e-20)
B_ISXY = -1.5 * LN2
LNK = LN2 / (1 << 23)
_C_MEAN = 1.5 - 1.0 / LN2
SIGMA2 = (127.0 - _C_MEAN) * (1 << 23) + (LN2 + LOG2PI) / LNK


def _register_dve_op(name: str, spec: Spec, subdim: bool = False):
    if name in dve_ops._SUB_OPCODE_FOR_NAME:
        return next(op for op in dve_ops.OPS if op.name == name)
    shas = {}
    for ver in ("v3", "v4"):
        uops = lower(spec, ver=ver)
        shas[ver] = DveOpSpec(
            name=name, opcode=0, uops=uops, rd1_en=_has_src1(spec)
        ).sha(ver)
    op = dve_ops.DveOp(name, spec, subdim=subdim, uops_sha=shas)
    dve_ops.OPS.append(op)
    dve_ops._SUB_OPCODE_FOR_NAME[name] = (
        dve_ops._CUSTOM_DVE_ROW_BASE + len(dve_ops.OPS) - 1
    )
    dve_ops.CUSTOM_DVE_SPECS[name] = spec
    return op


SQ2 = _register_dve_op(
    "SQ2_BGNLL",
    Spec(
        body=sq(Src0) + sq(Src1),
        reference=lambda in0, in1, s0, s1, imm2: (
            np.square(in0.astype(np.float32)) + np.square(in1.astype(np.float32))
        ).astype(np.float32),
    ),
)


def _ref_addmin_acc(in0, in1, s0, s1, imm2):
    b = np.minimum(in0.astype(np.float32) + in1.astype(np.float32), s0)
    b = b.astype(np.float32)
    return b, b.reshape(b.shape[0], -1).sum(axis=-1, keepdims=True)


# out = min(in0 + in1, s0); accum_out = sum(out)
ADDMIN_ACC = _register_dve_op(
    "ADDMINACC_BGNLL",
    Spec(
        body=minn(Src0 + Src1, C0),
        accum=AluOp.ADD,
        accum_init=Zero,
        reference=_ref_addmin_acc,
    ),
)


def _emit(ctx: ExitStack, tc: tile.TileContext, inp: bass.AP, loss: bass.AP):
    nc = tc.nc

    iot = ctx.enter_context(tc.tile_pool(name="iot", bufs=3))
    tp = ctx.enter_context(tc.tile_pool(name="tp", bufs=2))
    single = ctx.enter_context(tc.tile_pool(name="single", bufs=1))

    blocks = [(g, j * F, F) for g in range(G) for j in range(NF)]
    NBLK = len(blocks)

    acc = single.tile([128, NBLK], F32)
    lossall = single.tile([128, G], F32)

    inp3 = inp.rearrange("p (c f) -> p c f", c=7)
    ctxs: dict[int, dict] = {}

    def stage_load(blk):
        g, f0, fl = blocks[blk]
        view = inp3[g * 128:(g + 1) * 128, :, f0:f0 + fl]
        tin = iot.tile([128, 5, F], BF16, tag="in")
        # The [lsx|lsy|-pc] planes land first (they feed ScalarE and the
        # s1 chain); [x|muy] goes on the scalar HWDGE queue so it doesn't
        # serialize behind them, then the host-negated [-mux|-y] planes
        # accumulate onto it (CCE add) to form [nx | -ny] in the DMA.
        nc.sync.dma_start(tin[:, 0:2, :fl], view[:, 0:2, :])
        nc.gpsimd.dma_start(tin[:, 0:2, :fl], view[:, 2:4, :],
                            accum_op=ALU.add)
        nc.sync.dma_start(tin[:, 2:5, :fl], view[:, 4:7, :])
        ctxs[blk] = {"in": tin, "fl": fl}

    def stage_front(blk):
        c = ctxs[blk]
        tin = c["in"]
        fl = c["fl"]
        ls = tin[:, 2:4, :fl]
        pcv = tin[:, 4, :fl]

        # s1 = lsx + lsy on DVE, then -pc (host-negated plane) accumulates
        # onto it via one early SBUF->SBUF CCE-add DMA — no ScalarE
        # dependency, so the hop hides in the block skew.  (GpSimd tensor
        # ops would starve the DVE of SBUF bandwidth.)
        s1 = tp.tile([128, F], BF16, tag="s1")
        nc.vector.tensor_add(s1[:, :fl], ls[:, 0, :], ls[:, 1, :])
        nc.gpsimd.dma_start(s1[:, :fl], pcv, accum_op=ALU.add)
        c["s1"] = s1

        isxy = tp.tile([128, 2, F], BF16, tag="isxy")
        e1 = tp.tile([128, F], BF16, tag="e1")
        gm = tp.tile([128, F], BF16, tag="gm")
        nc.scalar.activation(isxy[:, :, :fl], ls, AF.Exp, scale=-1.0,
                             bias=B_ISXY)
        nc.scalar.activation(e1[:, :fl], pcv, AF.Exp, scale=-1.0)
        nc.scalar.activation(gm[:, :fl], pcv, AF.Exp, scale=2.0)

        hf = tp.tile([128, F], F32, tag="hf")
        hfb = tp.tile([128, F], BF16, tag="hfb")
        lvc = tp.tile([128, F], BF16, tag="lvc")
        # hF = 1 + e^{-2pc}: f32 copy for the bit-trick log, bf16 for the mul
        nc.scalar.activation(hf[:, :fl], gm[:, :fl], AF.Identity, scale=1.0,
                             bias=1.0)
        nc.scalar.activation(hfb[:, :fl], gm[:, :fl], AF.Identity, scale=1.0,
                             bias=1.0)
        # lvc = -(bits(hF) - SIGMA2)*LNK = -ln(hF) + ln2 + ln(2pi)
        nc.scalar.activation(lvc[:, :fl], hf[:, :fl].bitcast(I32), AF.Identity,
                             scale=-LNK, bias=SIGMA2 * LNK)
        c.update(isxy=isxy, e1=e1, hfb=hfb, lvc=lvc)

    def stage_main(blk):
        g, f0, fl = blocks[blk]
        c = ctxs[blk]
        tin = c["in"]
        xmy = tin[:, 0:2, :fl]

        nc.vector.tensor_mul(xmy, xmy, c["isxy"][:, :, :fl])   # [a | -b]
        mp = tp.tile([128, 2, F], BF16, tag="mp")
        a_ = tin[:, 0, :fl]
        bm_ = tin[:, 1, :fl]
        nc.vector.tensor_add(mp[:, 0, :fl], a_, bm_)     # m = a - b
        nc.vector.tensor_sub(mp[:, 1, :fl], a_, bm_)     # p2 = a + b
        nc.vector.tensor_mul(mp[:, 0, :fl], mp[:, 0, :fl],
                             c["e1"][:, :fl])            # mq
        q = tp.tile([128, F], BF16, tag="q")
        nc.vector._custom_dve(SQ2, out=q[:, :fl], in0=mp[:, 0, :fl],
                              in1=mp[:, 1, :fl])
        nc.vector.tensor_mul(q[:, :fl], q[:, :fl],
                             c["hfb"][:, :fl])           # W = (1+e^{-2pc})*Q
        nc.vector.tensor_add(c["s1"][:, :fl], c["s1"][:, :fl],
                             c["lvc"][:, :fl])           # s1b
        # nll = min(W + s1b, CLAMP); acc[:, blk] = sum_f nll
        nc.vector._custom_dve(
            ADDMIN_ACC, out=q[:, :fl], in0=q[:, :fl], in1=c["s1"][:, :fl],
            s0=CLAMP, accum_out=acc[:, blk:blk + 1],
        )
        del ctxs[blk]

    for i in range(NBLK + 2):
        if i < NBLK:
            stage_load(i)
        if 1 <= i and i - 1 < NBLK:
            stage_front(i - 1)
        if 2 <= i and i - 2 < NBLK:
            stage_main(i - 2)

    for g in range(G):
        cols = [i for i, b in enumerate(blocks) if b[0] == g]
        nc.vector.tensor_add(lossall[:, g:g + 1], acc[:, cols[0]:cols[0] + 1],
                             acc[:, cols[1]:cols[1] + 1])
        for i in cols[2:]:
            nc.vector.tensor_add(lossall[:, g:g + 1],
                                 lossall[:, g:g + 1], acc[:, i:i + 1])
    nc.sync.dma_start(loss, lossall[:])


_CACHED_NC = None


def _build_program() -> bass.Bass:
    global _CACHED_NC
    if _CACHED_NC is not None:
        return _CACHED_NC
    nc = bacc.Bacc("TRN2", target_bir_lowering=False, debug=False,
                   enable_asserts=False)
    for v in (0.0, B_ISXY, SIGMA2 * LNK):
        t = nc.alloc_sbuf_tensor(f"const-f32-{v}", [128, 1], F32)
        nc.gpsimd.memset(t.ap(), v)
        nc.const_aps.aps[(F32, v)] = t.ap()
    nc.all_engine_barrier()
    inp = nc.dram_tensor("inp", [PCC, 7 * T], BF16, kind="ExternalInput").ap()
    loss = nc.dram_tensor("loss", [128, G], F32, kind="ExternalOutput").ap()
    with tile.TileContext(nc) as tc:
        with ExitStack() as ctx:
            _emit(ctx, tc, inp, loss)
    nc.compile()
    _CACHED_NC = nc
    return nc


def make_in_maps(targets: np.ndarray, params: np.ndarray):
    bf = mybir.dt.np(BF16)
    targets = np.asarray(targets, dtype=np.float32)
    params = np.asarray(params, dtype=np.float32)
    in_maps = []
    for i in range(N_CORES):
        sl = slice(i * PCC, (i + 1) * PCC)
        arr = np.empty((PCC, 7, T), dtype=bf)
        arr[:, 0, :] = targets[:, sl, 0].T.astype(bf)
        arr[:, 1, :] = params[:, sl, 1].T.astype(bf)     # muy
        arr[:, 2, :] = (-params[:, sl, 0].T).astype(bf)  # -mux
        arr[:, 3, :] = (-targets[:, sl, 1].T).astype(bf) # -y
        arr[:, 4, :] = params[:, sl, 2].T.astype(bf)     # lsx
        arr[:, 5, :] = params[:, sl, 3].T.astype(bf)     # lsy
        arr[:, 6, :] = (-params[:, sl, 4].T).astype(bf)  # -pc
        in_maps.append({"inp": arr.reshape(PCC, 7 * T)})
    return in_maps


def run_spmd(targets: np.ndarray, params: np.ndarray, trace: bool = False):
    nc = _build_program()
    in_maps = make_in_maps(targets, params)
    res = bass_utils.run_bass_kernel_spmd(
        nc, in_maps, core_ids=list(range(N_CORES)), trace=trace,
    )
    loss = np.concatenate(
        [res.results[i]["loss"].reshape(128, G).T.ravel()
         for i in range(N_CORES)]
    ).astype(np.float32)
    return loss, res


def kernel(targets: np.ndarray, params: np.ndarray,
           peopleIDs: np.ndarray | None = None) -> np.ndarray:
    loss, _ = run_spmd(targets, params, trace=False)
    return loss

